# revision 2
# baseline (speedup 1.0000x reference)
"""BasisConv GNN message passing on 8 TRN2 NeuronCores — v5.

Device math (per 128-edge tile, all f16 with f32 PSUM accumulation):
  u[e,(a,i)] = bx[e,a] * f[e,i]           (DVE)
  uT         = PE transpose of u
  z[e,(c,o)] = uT.T @ W2                  (PE, 128-deep contraction)
  zz         = z * by[e,c]                (DVE)
  msg[e,o]   = sum_c zz[e,(c,o)]          (DVE reduce)
  seg[q,o]  += smat[:,q].T @ msg          (PE, PSUM quadrant per tile)
  scatter seg rows to out[node]           (indirect DMA, oob slots skipped)

Distribution: edges sorted by destination node, split into 8 contiguous
node ranges (collision-free outputs, no reduction needed).  The x_j table
is shipped as 1/8 shards and AllGathered on device.  All host->device
data rides in ONE u8 blob parameter per core (per-call parameter count
dominates the axon staging overhead).  A hardware For_i loop keeps the
program at ~200 instructions.
"""

import os
import sys

for _p in ("/opt/trn_rl_repo", "/opt/pypackages"):
    if _p not in sys.path:
        sys.path.insert(0, _p)

import time

import numpy as np

import jax

jax.config.update("jax_compilation_cache_dir", "/tmp/jaxcache")
jax.config.update("jax_persistent_cache_min_compile_time_secs", 0.0)

import concourse.bacc as bacc
import concourse.bass as bass
import concourse.mybir as mybir
import concourse.tile as tile
from concourse import bass_utils
from concourse.bass import ds

F = 32          # feature dim (in == out)
NB = 4          # basis terms per dimension
P = 128         # edges per tile
SEG = 32        # max segments (nodes) per tile
CH = 16         # tiles per chunk (one loop iteration)
GRP = 4         # tiles per PSUM column group
NG = CH // GRP
NCORES = 8
DX = 2.0 / (NB - 1)
CENTERS = np.linspace(-1.0, 1.0, NB, dtype=np.float32)
DUMMY_ATTR = 99.0
CB = 2 * P + 2 * NB + SEG     # consts columns (f16): W2 | ident | centers | iota
LAST_RESULTS = None
LAST_TIMES = None


def _pack_core(dst, n0, n1, e0, e1):
    n_range = n1 - n0
    counts = np.bincount(dst[e0:e1] - n0, minlength=n_range)
    tiles = []
    cur = []
    used = 0
    spares = []
    e = e0
    for ln in range(n_range):
        cnt = int(counts[ln])
        if cnt == 0:
            continue
        parts = []
        while cnt > P:
            parts.append(P)
            cnt -= P
        parts.append(cnt)
        for pi, pcnt in enumerate(parts):
            if pi == 0:
                row = ln
            else:
                row = n_range + len(spares)
                spares.append((ln, len(spares)))
            if used + pcnt > P or len(cur) >= SEG:
                tiles.append(cur)
                cur = []
                used = 0
            cur.append((row, e, pcnt))
            used += pcnt
            e += pcnt
    if cur:
        tiles.append(cur)
    return tiles, spares, n_range


def _build_device_arrays(tiles_list, ranges, src_g, attr_s, rows):
    """src_g: remapped uint16 gather indices (padded-shard node space)."""
    T = max(len(t) for t in tiles_list)
    T = ((T + CH - 1) // CH) * CH
    per_core = []
    for c in range(NCORES):
        tiles = tiles_list[c]
        src_il = np.zeros((P, T), np.uint16)
        attr_il = np.full((P, T, 2), DUMMY_ATTR, np.float16)
        seg_il = np.zeros((P, T), np.uint8)
        nid_il = np.full((P, T // GRP), rows, np.int32)   # sentinel = ROWS
        for t, nodes in enumerate(tiles):
            p = 0
            g, j = divmod(t, GRP)
            for q, (row, e_start, cnt) in enumerate(nodes):
                sl = slice(p, p + cnt)
                src_il[sl, t] = src_g[e_start:e_start + cnt]
                attr_il[sl, t, :] = attr_s[e_start:e_start + cnt]
                seg_il[sl, t] = q
                nid_il[SEG * j + q, g] = row
                p += cnt
        per_core.append({
            "src_il": src_il,
            "attr_il": np.ascontiguousarray(attr_il.reshape(P, T * 2)),
            "seg_il": seg_il,
            "nid_il": nid_il,
        })
    return per_core, T


def _build_nc(shp, T, ROWS):
    """shp: padded shard rows (shp*NCORES node slots, shp % P == 0)."""
    nc = bacc.Bacc("TRN2", target_bir_lowering=False, debug=False,
                   enable_asserts=False, num_devices=NCORES)
    f32, f16 = mybir.dt.float32, mybir.dt.float16
    i32, u8 = mybir.dt.int32, mybir.dt.uint8
    u16 = mybir.dt.uint16

    NC = T // CH
    # per-chunk edata block (bytes/partition):
    #   src u16 CH*2 | attr f16 CH*2*2 | seg u8 CH | nid i32 NG*4
    EB = CH * 2 + CH * 4 + CH + NG * 4
    XB = shp * F * 2 // P              # xjs shard bytes per partition
    BW = XB + NC * EB + CB * 2
    blob_d = nc.dram_tensor("blob", [P, BW], u8, kind="ExternalInput")
    out_d = nc.dram_tensor("out", [ROWS, F], f16, kind="ExternalOutput")

    with tile.TileContext(nc) as tc:
        with (
            tc.tile_pool(name="dram", bufs=1, space="DRAM") as dpool,
            tc.tile_pool(name="const", bufs=1) as cpool,
            tc.tile_pool(name="io", bufs=2) as iopool,
            tc.tile_pool(name="idx32", bufs=2) as xpool,
            tc.tile_pool(name="feat", bufs=2) as fpool,
            tc.tile_pool(name="basis", bufs=2) as bpool,
            tc.tile_pool(name="u", bufs=3) as upool,
            tc.tile_pool(name="ut", bufs=3) as utpool,
            tc.tile_pool(name="zz", bufs=3) as zzpool,
            tc.tile_pool(name="msg", bufs=3) as mpool,
            tc.tile_pool(name="stage", bufs=2) as stpool,
            tc.tile_pool(name="utp", bufs=2, space="PSUM") as utps,
            tc.tile_pool(name="zp", bufs=2, space="PSUM") as zps,
            tc.tile_pool(name="sp", bufs=2, space="PSUM") as sps,
        ):
            # x_j shard -> DRAM bounce -> AllGather to full table
            xg_in = dpool.tile([shp, F], f16, tag="xgin")
            xj_full = dpool.tile([NCORES * shp, F], f16, tag="xgout")
            nc.gpsimd.dma_start(
                xg_in[:].rearrange("(a b) f -> a (b f)", a=P),
                blob_d[:, 0:XB].bitcast(f16))
            nc.gpsimd.collective_compute(
                "AllGather",
                mybir.AluOpType.bypass,
                replica_groups=[list(range(NCORES))],
                ins=[xg_in.opt()],
                outs=[xj_full.opt()],
            )

            cst = cpool.tile([P, CB], f16, tag="cst")
            nc.sync.dma_start(cst[:], blob_d[:, XB + NC * EB:BW].bitcast(f16))
            wst = cst[:, 0:P]
            ident = cst[:, P:2 * P]
            cen = cst[:, 2 * P:2 * P + 2 * NB]
            io32 = cst[:, 2 * P + 2 * NB:]

            with tc.For_i(0, NC, 1) as ci:
                ed = iopool.tile([P, EB], u8, tag="ed")
                nc.sync.dma_start(ed[:], blob_d[:, ds(ci * EB + XB, EB)])
                idx16 = ed[:, 0:CH * 2].bitcast(u16)
                attr = ed[:, CH * 2:CH * 6].bitcast(f16)
                segu = ed[:, CH * 6:CH * 7]
                nidt = ed[:, CH * 7:EB].bitcast(i32)

                idx = xpool.tile([P, CH], i32, tag="idx")
                nc.vector.tensor_copy(out=idx[:], in_=idx16)

                feat = fpool.tile([P, CH * F], f16, tag="feat")
                for tl in range(CH):
                    nc.gpsimd.indirect_dma_start(
                        out=feat[:, tl * F:(tl + 1) * F],
                        out_offset=None, in_=xj_full[:],
                        in_offset=bass.IndirectOffsetOnAxis(
                            ap=idx[:, tl:tl + 1], axis=0))

                # segment one-hot S[p,t,q] = (seg[p,t] == q), f16 0/1
                segf = bpool.tile([P, CH], f16, tag="segf")
                nc.vector.tensor_copy(out=segf[:], in_=segu)
                smat = bpool.tile([P, CH * SEG], f16, tag="smat")
                nc.vector.tensor_tensor(
                    out=smat[:].rearrange("p (t q) -> p t q", t=CH),
                    in0=segf[:].unsqueeze(2).to_broadcast([P, CH, SEG]),
                    in1=io32.unsqueeze(1).to_broadcast([P, CH, SEG]),
                    op=mybir.AluOpType.is_equal)

                # hat basis for the whole chunk: [P, CH, 2, NB]
                bxy = bpool.tile([P, CH * 2 * NB], f16, tag="bxy")
                bxy_v = bxy[:].rearrange("p (t d n) -> p t d n", t=CH, d=2)
                nc.vector.tensor_tensor(
                    out=bxy_v,
                    in0=attr.rearrange("p (t d) -> p t d", d=2)
                        .unsqueeze(3).to_broadcast([P, CH, 2, NB]),
                    in1=cen.rearrange("p (d n) -> p d n", d=2)
                        .unsqueeze(1).to_broadcast([P, CH, 2, NB]),
                    op=mybir.AluOpType.subtract)
                nc.scalar.activation(
                    out=bxy[:], in_=bxy[:],
                    func=mybir.ActivationFunctionType.Abs,
                    scale=1.0 / DX)
                nc.scalar.activation(
                    out=bxy[:], in_=bxy[:],
                    func=mybir.ActivationFunctionType.Relu,
                    bias=1.0, scale=-1.0)

                for g in range(NG):
                    seg_ps = sps.tile([P, F], f32, tag="sps")
                    gsl = slice(g * GRP * 8, (g + 1) * GRP * 8)
                    bxy_g = bxy[:, gsl].rearrange(
                        "p (t d n) -> p t d n", t=GRP, d=2)
                    # u for 4 tiles in one DVE op: [P, (t,a,i)]
                    u4 = upool.tile([P, GRP * P], f16, tag="u")
                    nc.vector.tensor_tensor(
                        out=u4[:].rearrange("p (t a i) -> p t a i", t=GRP, a=NB),
                        in0=feat[:, g * GRP * F:(g + 1) * GRP * F]
                            .rearrange("p (t i) -> p t i", t=GRP)
                            .unsqueeze(2).to_broadcast([P, GRP, NB, F]),
                        in1=bxy_g[:, :, 0, :]
                            .unsqueeze(3).to_broadcast([P, GRP, NB, F]),
                        op=mybir.AluOpType.mult)
                    z4 = zps.tile([P, GRP * P], f32, tag="zps")
                    for j in range(GRP):
                        ut_ps = utps.tile([P, P], f16, tag="utps")
                        nc.tensor.transpose(
                            out=ut_ps[:], in_=u4[:, j * P:(j + 1) * P],
                            identity=ident)
                        ut_sb = utpool.tile([P, P], f16, tag="ut")
                        nc.scalar.copy(out=ut_sb[:], in_=ut_ps[:])
                        nc.tensor.matmul(
                            out=z4[:, j * P:(j + 1) * P], lhsT=ut_sb[:],
                            rhs=wst, start=True, stop=True)
                    # zz and c-sum for 4 tiles in one op each
                    zz4 = zzpool.tile([P, GRP * P], f16, tag="zz")
                    nc.vector.tensor_tensor(
                        out=zz4[:].rearrange("p (t c o) -> p t c o", t=GRP, c=NB),
                        in0=z4[:].rearrange("p (t c o) -> p t c o", t=GRP, c=NB),
                        in1=bxy_g[:, :, 1, :]
                            .unsqueeze(3).to_broadcast([P, GRP, NB, F]),
                        op=mybir.AluOpType.mult)
                    msg4 = mpool.tile([P, GRP * F], f16, tag="msg")
                    with nc.allow_low_precision("4-term c-sum in f16"):
                        nc.vector.tensor_reduce(
                            out=msg4[:],
                            in_=zz4[:].rearrange("p (t c o) -> p t o c", t=GRP, c=NB),
                            axis=mybir.AxisListType.X,
                            op=mybir.AluOpType.add)
                    for j in range(GRP):
                        tl = g * GRP + j
                        nc.tensor.matmul(
                            out=seg_ps[SEG * j:SEG * (j + 1), :],
                            lhsT=smat[:, tl * SEG:(tl + 1) * SEG],
                            rhs=msg4[:, j * F:(j + 1) * F],
                            start=True, stop=True,
                            skip_group_check=True,
                            tile_position=(0, SEG * j))
                    stage = stpool.tile([P, F], f16, tag="stage")
                    nc.scalar.copy(out=stage[:], in_=seg_ps[:])
                    nc.gpsimd.indirect_dma_start(
                        out=out_d[:, :],
                        out_offset=bass.IndirectOffsetOnAxis(
                            ap=nidt[:, g:g + 1], axis=0),
                        in_=stage[:], in_offset=None,
                        bounds_check=ROWS - 1, oob_is_err=False)

    nc.compile()
    return nc


def kernel(x_i, x_j, edge_index, edge_attr, weight):
    n_nodes = int(np.asarray(x_i).shape[0])
    x_j16 = np.asarray(x_j, np.float16)
    ei = np.asarray(edge_index)
    dst = ei[0].astype(np.int64)
    src = ei[1].astype(np.int64)
    attr = np.asarray(edge_attr, np.float16)
    w = np.asarray(weight, np.float32)
    E = dst.shape[0]

    sh = (n_nodes + NCORES - 1) // NCORES           # true shard rows
    shp = ((sh + P - 1) // P) * P                   # padded to 128 rows
    assert sh * NCORES >= n_nodes

    order = np.argsort(dst, kind="stable")
    dst_s = dst[order]
    src_s = src[order]
    attr_s = attr[order]
    # remap source ids into the padded-shard node space
    src_g = ((src_s // sh) * shp + (src_s % sh)).astype(np.uint16) \
        if NCORES * shp < 65536 else \
        ((src_s // sh) * shp + (src_s % sh)).astype(np.uint32)
    assert NCORES * shp < 65536, "u16 gather ids require < 64k padded nodes"

    counts = np.bincount(dst_s, minlength=n_nodes)
    cume = np.concatenate([[0], np.cumsum(counts)])
    node_bounds = [0]
    for c in range(1, NCORES):
        node_bounds.append(int(np.searchsorted(cume, E * c // NCORES)))
    node_bounds.append(n_nodes)

    tiles_list, spares_list, ranges = [], [], []
    for c in range(NCORES):
        n0, n1 = node_bounds[c], node_bounds[c + 1]
        e0, e1 = int(cume[n0]), int(cume[n1])
        tiles, spares, n_range = _pack_core(dst_s, n0, n1, e0, e1)
        tiles_list.append(tiles)
        spares_list.append(spares)
        ranges.append(n_range)

    n_spare = max((len(s) for s in spares_list), default=0)
    ROWS = max(ranges) + n_spare
    per_core, T = _build_device_arrays(tiles_list, ranges, src_g, attr_s, ROWS)
    NC = T // CH
    EB = CH * 2 + CH * 4 + CH + NG * 4
    XB = shp * F * 2 // P

    # consts block
    wstack = np.ascontiguousarray(
        w.transpose(0, 2, 1, 3).reshape(P, P)).astype(np.float16)
    ident = np.eye(P, dtype=np.float16)
    cen8 = np.tile(np.concatenate([CENTERS, CENTERS])[None, :],
                   (P, 1)).astype(np.float16)
    io32 = np.tile(np.arange(SEG, dtype=np.float16)[None, :], (P, 1))
    consts = np.concatenate([wstack, ident, cen8, io32], axis=1)  # [P, CB] f16

    # shard table, zero-padded to shp rows per core
    xpad = np.zeros((NCORES * shp, F), np.float16)
    for c in range(NCORES):
        lo, hi = c * sh, min((c + 1) * sh, n_nodes)
        xpad[c * shp:c * shp + (hi - lo)] = x_j16[lo:hi]

    nc = _build_nc(shp, T, ROWS)

    in_maps = []
    for c in range(NCORES):
        m = per_core[c]
        blob = np.empty((P, XB + NC * EB + CB * 2), np.uint8)
        blob[:, 0:XB] = xpad[c * shp:(c + 1) * shp].view(np.uint8).reshape(P, XB)
        eb = blob[:, XB:XB + NC * EB].reshape(P, NC, EB)
        eb[:, :, 0:CH * 2] = np.ascontiguousarray(
            m["src_il"].reshape(P, NC, CH)).view(np.uint8)
        eb[:, :, CH * 2:CH * 6] = np.ascontiguousarray(
            m["attr_il"].reshape(P, NC, CH * 2)).view(np.uint8)
        eb[:, :, CH * 6:CH * 7] = m["seg_il"].reshape(P, NC, CH)
        eb[:, :, CH * 7:EB] = np.ascontiguousarray(
            m["nid_il"].reshape(P, NC, NG)).view(np.uint8)
        blob[:, XB + NC * EB:] = consts.view(np.uint8).reshape(P, CB * 2)
        in_maps.append({"blob": blob})

    res = bass_utils.run_bass_kernel_spmd(nc, in_maps, core_ids=list(range(NCORES)))
    global LAST_RESULTS, LAST_TIMES
    LAST_RESULTS = res
    if os.environ.get("BC_TIME_REPEATS"):
        times = []
        for _ in range(int(os.environ["BC_TIME_REPEATS"])):
            t0 = time.time()
            bass_utils.run_bass_kernel_spmd(nc, in_maps, core_ids=list(range(NCORES)))
            times.append(time.time() - t0)
        LAST_TIMES = times

    out = np.zeros((n_nodes, F), np.float32)
    for c in range(NCORES):
        r = res.results[c]["out"].astype(np.float32)      # [ROWS, F]
        n0 = node_bounds[c]
        n_range = ranges[c]
        out[n0:n0 + n_range] = r[:n_range]
        for true_ln, si in spares_list[c]:
            out[n0 + true_ln] += r[n_range + si]
    return out
